# revision 26
# baseline (speedup 1.0000x reference)
"""Head-parallel HGNN attention-coefficient kernel for Trainium2 (Bass/Tile).

Per head h (8 heads):
    Q = emb_dest @ Wq[h] + bq[h]            [4096, 512]
    K = emb_src  @ Wk[h] + bk[h]            [4096, 512]
    V = feat_src @ Wv[h] + bv[h]            [4096, 512]
    S = Q @ K^T / sqrt(512)                 [4096, 4096]
    O = elu(softmax(S, -1) @ V)             [4096, 512]
output = mean_h O                           [4096, 512]

One head per NeuronCore; no collectives; host averages the 8 outputs.

All O(N^2) matmuls (and the Q/K projections) run in fp8e4
DoubleRowSwInterleave (256-deep contraction per instruction, ~129ns on
HW vs 169ns for plain DoubleRow: the pre-interleaved weight layout
keeps the fast weight-load path).  The stationary operands are produced
directly in the interleaved-reversed layout: Wq/Wk are interleaved on
the host; K^T and exp(S^T) are written interleaved on-device via
negative-stride APs.  The softmax denominator is a ones-vector plain-DR
matmul over a stride-2 plane view of the interleaved exp tiles (it sums
exactly the fp8 values P@V consumes, so softmax stays consistent; the
resulting block-reversed order is undone in the PSUM->SBUF copy).
V projection stays f16 for accuracy (fp8 everywhere else would push
the error over the 2e-2 budget).

The instruction stream is software-pipelined: the scores/exp production
of strip s+1 is interleaved slot-by-slot with the P@V consumption of
strip s, so the in-order PE and ScalarE queues overlap instead of
alternating.  Q-projection runs two strips ahead inside the steady
loop; the V projection fills period 0's idle PE window; the K
projection is the prologue.  Measured: 216us HW (vs 596us f16
baseline), final rel err 1.71e-2 (budget 2e-2).
"""

import numpy as np

P = 128
D = 512            # IN_DIM
E = 512            # HIDDEN
N = 4096           # N_DST
M = 4096           # N_SRC
H = 8
DC = D // P        # 4 contraction chunks for projections
EC = E // P        # 4
MC = M // P        # 32 N_src chunks
MCP = MC // 2      # 16 N_src chunk pairs (DoubleRow)
NSTRIP = 512       # N_dst columns handled per strip
NSTRIPS = N // NSTRIP
NCH = NSTRIP // P  # 4 N_dst chunks per strip
WSCALE = 16.0      # host pre-scale on Wq/Wk/bq/bk (fp8 subnormal dodge)
SCALE = 1.0 / (float(np.sqrt(E)) * WSCALE * WSCALE)

_cache = {}


def _build_nc(repeat=1):
    import concourse.mybir as mybir
    import concourse.tile as tile
    from concourse import bacc

    f32 = mybir.dt.float32
    f16 = mybir.dt.float16
    f8 = mybir.dt.float8e4
    AF = mybir.ActivationFunctionType
    ALU = mybir.AluOpType
    DRI = mybir.MatmulPerfMode.DoubleRowSwInterleave
    DR = mybir.MatmulPerfMode.DoubleRow

    nc = bacc.Bacc(
        "TRN2",
        target_bir_lowering=False,
        debug=False,
        enable_asserts=False,
        num_devices=H,
    )

    embT_d_h = nc.dram_tensor("embT_dest", [D, N], f8, kind="ExternalInput")
    embT_s_h = nc.dram_tensor("embT_src", [D, M], f8, kind="ExternalInput")
    featT_h = nc.dram_tensor("featT_src", [E, M], f16, kind="ExternalInput")
    # host-interleaved DRI weights: [p, dcp, ec, 2*(127-u)+i]
    wq_h = nc.dram_tensor("Wqi", [P, DC // 2, EC * 2 * P], f8, kind="ExternalInput")
    wk_h = nc.dram_tensor("Wki", [P, DC // 2, EC * 2 * P], f8, kind="ExternalInput")
    wv_h = nc.dram_tensor("Wv", [E, E], f16, kind="ExternalInput")
    bq_h = nc.dram_tensor("bq", [E], f32, kind="ExternalInput")
    bk_h = nc.dram_tensor("bk", [E], f32, kind="ExternalInput")
    bv_h = nc.dram_tensor("bv", [E], f16, kind="ExternalInput")
    out_h = nc.dram_tensor("out", [N, E], f32, kind="ExternalOutput")

    embT_d = embT_d_h.ap().rearrange("(c p) n -> p c n", p=P)
    embT_s = embT_s_h.ap().rearrange("(c p) n -> p c n", p=P)
    featT = featT_h.ap().rearrange("(c p) n -> p c n", p=P)
    out_ap = out_h.ap()

    with tile.TileContext(nc) as tc:
        with (
            tc.tile_pool(name="wpool", bufs=1) as wpool,
            tc.tile_pool(name="cpool", bufs=1) as cpool,
            tc.tile_pool(name="big", bufs=1) as big_pool,
            tc.tile_pool(name="embx", bufs=4) as embx_pool,
            tc.tile_pool(name="pt", bufs=32) as pt_pool,
            tc.tile_pool(name="ep", bufs=4) as ep_pool,
            tc.tile_pool(name="sm", bufs=2) as sm_pool,
            tc.tile_pool(name="psA", bufs=5, space="PSUM") as psA,
            tc.tile_pool(name="psO", bufs=2, space="PSUM") as psO,
            tc.tile_pool(name="psSm", bufs=1, space="PSUM") as psSm,
        ):
            # --- constants / weights ---
            wq_sb = wpool.tile([P, DC // 2, EC, 2 * P], f8, name="wq_sb")
            nc.sync.dma_start(
                wq_sb[:], wq_h.ap().rearrange("p c (e u) -> p c e u", e=EC)
            )
            wk_sb = wpool.tile([P, DC // 2, EC, 2 * P], f8, name="wk_sb")
            nc.sync.dma_start(
                wk_sb[:], wk_h.ap().rearrange("p c (e u) -> p c e u", e=EC)
            )
            wv_sb = wpool.tile([P, EC, E], f16, name="wv_sb")
            nc.sync.dma_start(wv_sb[:], wv_h.ap().rearrange("(c p) e -> p c e", p=P))
            bq_sb = cpool.tile([P, EC], f32, name="bq_sb")
            nc.sync.dma_start(bq_sb[:], bq_h.ap().rearrange("(c p) -> p c", p=P))
            bk_sb = cpool.tile([P, EC], f32, name="bk_sb")
            nc.sync.dma_start(bk_sb[:], bk_h.ap().rearrange("(c p) -> p c", p=P))
            bv_sb = cpool.tile([1, E], f16, name="bv_sb")
            nc.sync.dma_start(bv_sb[:], bv_h.ap().rearrange("(o e) -> o e", o=1))

            ones_row = cpool.tile([1, P], f16, name="ones_row")
            nc.any.memset(ones_row[:], 1.0)
            # fp8 ones pair for the DR denominator matmul (pair stride 16B)
            ones8 = cpool.tile([P, 2, 16], f8, name="ones8")
            nc.any.memset(ones8[:], 1.0)
            one_one = cpool.tile([1, 1], f32, name="one_one")
            nc.any.memset(one_one[:], 1.0)

            for _rep in range(repeat):
                # --- persistent activations ---
                qt_sb = big_pool.tile([P, EC, N], f8, tag="qt", name="qt_sb")
                # interleaved K^T, one tile per ec-pair: [p, mc, 2*(127-u)+i]
                kt_i = [
                    big_pool.tile([P, MC, 2 * P], f8, tag=f"kt{ecp}", name=f"kt{ecp}")
                    for ecp in range(EC // 2)
                ]
                v_sb = big_pool.tile([P, MC, E], f8, tag="v", name="v_sb")

                def q_proj(nt):
                    """Q^T strip nt: DRI matmuls + DVE bias-add (fp8 store)."""
                    et = embx_pool.tile([P, DC, NSTRIP], f8, tag="embq", name="etq")
                    nc.sync.dma_start(
                        et[:], embT_d[:, :, nt * NSTRIP : (nt + 1) * NSTRIP]
                    )
                    for ec in range(EC):
                        ps = psA.tile([P, NSTRIP], f32, tag="psA", name="psq")
                        for dcp in range(DC // 2):
                            nc.tensor.matmul(
                                ps[:],
                                lhsT=wq_sb[:, dcp, ec, :],
                                rhs=et[:, 2 * dcp : 2 * dcp + 2, :],
                                start=(dcp == 0),
                                stop=(dcp == DC // 2 - 1),
                                perf_mode=DRI,
                            )
                        nc.vector.tensor_scalar_add(
                            qt_sb[:, ec, nt * NSTRIP : (nt + 1) * NSTRIP],
                            ps[:],
                            bq_sb[:, ec : ec + 1],
                        )

                def v_proj_chunk(mc, ft):
                    """V chunk mc: f16 matmuls + rank-1 bias, fp8 store."""
                    mi = mc % (NSTRIP // P)
                    ps = psA.tile([P, E], f32, tag="psA", name="psv")
                    for ec in range(EC):
                        nc.tensor.matmul(
                            ps[:],
                            lhsT=ft[:, ec, mi * P : (mi + 1) * P],
                            rhs=wv_sb[:, ec, :],
                            start=(ec == 0),
                            stop=False,
                        )
                    nc.tensor.matmul(
                        ps[:], lhsT=ones_row[:], rhs=bv_sb[:], start=False, stop=True
                    )
                    # split V stores between ScalarE and DVE
                    if mc % 2 == 0:
                        nc.scalar.activation(v_sb[:, mc, :], ps[:], AF.Copy)
                    else:
                        nc.vector.tensor_copy(v_sb[:, mc, :], ps[:])

                # ---------- prologue: K^T (interleaved) ----------
                for nt in range(M // NSTRIP):
                    et = embx_pool.tile([P, DC, NSTRIP], f8, tag="embq", name="etk")
                    nc.sync.dma_start(
                        et[:], embT_s[:, :, nt * NSTRIP : (nt + 1) * NSTRIP]
                    )
                    for ecp in range(EC // 2):
                        for j in range(2):
                            ec = 2 * ecp + j
                            ps = psA.tile([P, NCH, P], f32, tag="psA", name="psk")
                            for dcp in range(DC // 2):
                                nc.tensor.matmul(
                                    ps[:],
                                    lhsT=wk_sb[:, dcp, ec, :],
                                    rhs=et[:, 2 * dcp : 2 * dcp + 2, :],
                                    start=(dcp == 0),
                                    stop=(dcp == DC // 2 - 1),
                                    perf_mode=DRI,
                                )
                            # interleaved-reversed store: addr = 2*(127-u)+j
                            nc.scalar.activation(
                                kt_i[ecp][
                                    :, nt * NCH : (nt + 1) * NCH, (2 * P - 2 + j) :: -2
                                ],
                                ps[:],
                                AF.Identity,
                                bias=bk_sb[:, ec : ec + 1],
                            )

                q_proj(0)
                q_proj(1)

                # ---------- software-pipelined strip loop ----------
                # state carried between periods
                pts_prev = None      # pt tiles of strip s (consumed by PV)
                rinv_prev = None     # 1/denominator of strip s

                for period in range(NSTRIPS + 1):
                    sp = period          # strip whose scores/exp are produced
                    sc = period - 1      # strip whose PV/ELU are consumed
                    n0 = sp * NSTRIP
                    do_scores = sp < NSTRIPS
                    do_pv = sc >= 0

                    pts_new = []
                    cs_ps = (
                        psSm.tile([1, NSTRIP], f32, tag="sm", name="cs_ps")
                        if do_scores
                        else None
                    )
                    pos = [psO.tile([P, E], f32, tag="psO", name=f"po{k}") for k in range(2)] if do_pv else None

                    def emit_denom(mcp):
                        # denominator: DRI ones-matmul over the stride-2 plane
                        # view of the interleaved exp tile (same fp8 values
                        # the P@V matmul consumes)
                        nc.tensor.matmul(
                            cs_ps[:],
                            lhsT=ones8[:, :, 0:1],
                            rhs=pts_new[mcp][:, :, 0:NSTRIP],
                            start=(mcp == 0),
                            stop=(mcp == MCP - 1),
                            perf_mode=DR,
                        )

                    for k in range(MCP):
                        if do_scores:
                            mcp = k
                            # plane-contiguous exp tile; 2048B pair stride
                            # keeps the plain-DR weight fast path
                            ptt = pt_pool.tile([P, 2, 2 * NSTRIP], f8, tag="pt", name="ptt")
                            for j in range(2):
                                mc = 2 * mcp + j
                                ps = psA.tile([P, NSTRIP], f32, tag="psA", name="pss")
                                for ecp in range(2):
                                    nc.tensor.matmul(
                                        ps[:],
                                        lhsT=kt_i[ecp][:, mc, :],
                                        rhs=qt_sb[
                                            :, 2 * ecp : 2 * ecp + 2, n0 : n0 + NSTRIP
                                        ],
                                        start=(ecp == 0),
                                        stop=(ecp == 1),
                                        perf_mode=DRI,
                                    )
                                nc.scalar.activation(
                                    ptt[:, j, 0:NSTRIP],
                                    ps[:],
                                    AF.Exp,
                                    scale=SCALE,
                                )
                            pts_new.append(ptt)
                            # lag the denominator matmul 2 pairs behind the
                            # exps so the in-order PE never waits on ScalarE
                            if k >= 2:
                                emit_denom(k - 2)

                        if do_pv:
                            ncn = k // NCH
                            po = pos[ncn % 2]
                            for mm in range(NCH):
                                mcp = (k % NCH) * NCH + mm
                                nc.tensor.matmul(
                                    po[:],
                                    lhsT=pts_prev[mcp][:, :, ncn * P : (ncn + 1) * P],
                                    rhs=v_sb[:, 2 * mcp : 2 * mcp + 2, :],
                                    start=(mcp == 0),
                                    stop=(mcp == MCP - 1),
                                    perf_mode=DR,
                                )
                            if k % NCH == NCH - 1:
                                # normalize + ELU:
                                # elu(x) = max(x,0) + min(exp(x)-1, 0),
                                # x = po * rinv (per-partition scale)
                                rv = rinv_prev[:, ncn : ncn + 1]
                                ex = ep_pool.tile([P, E], f32, tag="ex", name="ex")
                                nc.scalar.activation(ex[:], po[:], AF.Exp, scale=rv)
                                t0 = ep_pool.tile([P, E], f32, tag="t0", name="t0")
                                nc.vector.tensor_scalar(
                                    t0[:], po[:], rv, 0.0, ALU.mult, ALU.max
                                )
                                nc.vector.tensor_scalar(
                                    ex[:], ex[:], -1.0, 0.0, ALU.add, ALU.min
                                )
                                nc.vector.tensor_add(t0[:], t0[:], ex[:])
                                nc.sync.dma_start(
                                    out_ap[
                                        sc * NSTRIP + ncn * P : sc * NSTRIP + (ncn + 1) * P,
                                        :,
                                    ],
                                    t0[:],
                                )

                        if period == 0:
                            # V projection fills period 0's idle PE window
                            if k % 2 == 0:
                                ft = embx_pool.tile(
                                    [P, EC, NSTRIP], f16, tag="embv", name="ft"
                                )
                                nc.sync.dma_start(
                                    ft[:],
                                    featT[:, :, (k // 2) * NSTRIP : (k // 2 + 1) * NSTRIP],
                                )
                            v_proj_chunk(2 * k, ft)
                            v_proj_chunk(2 * k + 1, ft)

                        if k == 7 and sp + 2 < NSTRIPS:
                            q_proj(sp + 2)

                    if do_scores:
                        emit_denom(MCP - 2)
                        emit_denom(MCP - 1)
                        # cs_raw2[b, f] holds denom of n = b*128 + (127-f):
                        # un-reverse while copying PSUM -> SBUF
                        cs_sb = sm_pool.tile([1, NSTRIP], f32, tag="cs_sb", name="cs_sb")
                        nc.vector.tensor_copy(cs_sb[:], cs_ps[:])
                        rt_ps = psSm.tile([P, NCH], f32, tag="sm", name="rt_ps")
                        for ncn in range(NCH):
                            nc.tensor.matmul(
                                rt_ps[:, ncn : ncn + 1],
                                lhsT=cs_sb[0:1, ncn * P : (ncn + 1) * P],
                                rhs=one_one[:],
                                start=True,
                                stop=True,
                            )
                        rinv = sm_pool.tile([P, NCH], f32, tag="rinv", name="rinv")
                        nc.vector.reciprocal(rinv[:], rt_ps[:])
                        rinv_prev = rinv
                        pts_prev = pts_new

    nc.compile()
    return nc


def _get_nc():
    nc = _cache.get("nc")
    if nc is None:
        nc = _build_nc()
        _cache["nc"] = nc
    return nc


def _interleave_w(w):
    """[D, E] -> DRI layout [p, dcp, ec*256 + 2*(127-u)+i]."""
    import ml_dtypes

    D_, E_ = w.shape
    wr = w.reshape(DC // 2, 2, P, EC, P)          # [dcp, i, p, ec, u]
    wr = wr[:, :, :, :, ::-1]                     # u -> 127-u
    wr = wr.transpose(2, 0, 3, 4, 1)              # [p, dcp, ec, u', i]
    return np.ascontiguousarray(wr.reshape(P, DC // 2, EC * 2 * P)).astype(
        ml_dtypes.float8_e4m3
    )


def _make_in_maps(inputs):
    import ml_dtypes

    f8 = ml_dtypes.float8_e4m3
    bf = np.float16
    f32 = np.float32
    embT_d = np.asarray(inputs["emb_dest"], f32).T.astype(f8)
    embT_s = np.asarray(inputs["emb_src"], f32).T.astype(f8)
    featT = np.asarray(inputs["feat_src"], f32).T.astype(bf)
    Wq = np.asarray(inputs["Wq"], f32) * WSCALE
    Wk = np.asarray(inputs["Wk"], f32) * WSCALE
    Wv = np.asarray(inputs["Wv"], f32)
    bq = np.asarray(inputs["bq"], f32) * WSCALE
    bk = np.asarray(inputs["bk"], f32) * WSCALE
    bv = np.asarray(inputs["bv"], f32)
    in_maps = []
    for h in range(H):
        in_maps.append(
            {
                "embT_dest": embT_d,
                "embT_src": embT_s,
                "featT_src": featT,
                "Wqi": _interleave_w(Wq[h]),
                "Wki": _interleave_w(Wk[h]),
                "Wv": Wv[h].astype(bf),
                "bq": np.ascontiguousarray(bq[h]),
                "bk": np.ascontiguousarray(bk[h]),
                "bv": bv[h].astype(bf),
            }
        )
    return in_maps


def kernel(**inputs):
    from concourse.bass_utils import run_bass_kernel_spmd

    nc = _get_nc()
    in_maps = _make_in_maps(inputs)
    res = run_bass_kernel_spmd(nc, in_maps, core_ids=list(range(H)))
    outs = np.stack([r["out"] for r in res.results], axis=0)
    return outs.mean(axis=0, dtype=np.float64).astype(np.float32)
